# revision 1
# baseline (speedup 1.0000x reference)
"""Trainium2 Bass kernel for CustomGPT2MultiHeadAttention (B=4, S=1024, SI=512,
D=1024, 16 heads), sharded over 8 NeuronCores.

Sharding: core c handles (batch b = c//2, head-group hg = c%2 of 8 heads).
Tensor-parallel on heads for QKV/attention; after the (per-core partial)
output projection, a pairwise ReduceScatter over {2b, 2b+1} produces disjoint
sequence halves of the final output, which the host concatenates.

Device-side math per core:
  qT[o,s]  = (w_q[hg]/8) @ hidden[b]^T      (fp32r matmuls, f32 PSUM accum)
  kT[o,k'] = w_k[hg] @ hidden[b]^T  ++  u_k[hg] @ image[b]^T
  v[k',o]  = (hidden[b] ++ image[b]) @ w_v/u_v[hg]^T   (natural layout, bf16)
  per head: scoresT[k',q] = kT^T-slice . qT-slice  (K=64 contraction)
            pT = exp(scoresT) * maskT             (no max-subtraction needed:
                                                   scores ~ N(0,1), exp safe)
            xT_aug[65,q] += [v | 1]^T . pT        (row 64 = masked softmax sums)
            xT[d,q] = xT_aug[0:64] * (1/sums)     (partition-broadcast recip)
  y_part[s,o] = xT^T . w_o^T[d-slice]             (bf16, partial over d)
  ReduceScatter(add) over the core pair -> y half [512, 1024] per core.
"""

import numpy as np
import ml_dtypes

import concourse.bass as bass
import concourse.bacc as bacc
import concourse.mybir as mybir
import concourse.tile as tile
from concourse import bass_utils

F32 = mybir.dt.float32
F32R = mybir.dt.float32r
BF16 = mybir.dt.bfloat16
I32 = mybir.dt.int32

D = 1024          # model dim
S = 1024          # text sequence
SI = 512          # image sequence
SK = S + SI       # 1536 keys
HL = 8            # heads per core
DH = 64           # head dim
P = 128
KT = SK // P      # 12 key tiles
OC = HL * DH      # 512 = per-core projection output dim

_CACHE = {}


def _build_nc(analysis=False, stop_after=None, rs_chunks=4):
    nc = bacc.Bacc("TRN2", target_bir_lowering=False, debug=False, num_devices=8)

    hT = nc.dram_tensor("hT", [D, S], F32, kind="ExternalInput")
    iT = nc.dram_tensor("iT", [D, SI], F32, kind="ExternalInput")
    mT = nc.dram_tensor("mT", [SK, S], I32, kind="ExternalInput")
    wq = nc.dram_tensor("wq", [D, OC], F32, kind="ExternalInput")
    wk = nc.dram_tensor("wk", [D, OC], F32, kind="ExternalInput")
    wv = nc.dram_tensor("wv", [D, OC], F32, kind="ExternalInput")
    uk = nc.dram_tensor("uk", [D, OC], F32, kind="ExternalInput")
    uv = nc.dram_tensor("uv", [D, OC], F32, kind="ExternalInput")
    wo = nc.dram_tensor("wo", [OC, D], F32, kind="ExternalInput")
    y = nc.dram_tensor("y", [S // 2, D], F32, kind="ExternalOutput")

    with tile.TileContext(nc) as tc:
        _body(tc, hT, iT, mT, wq, wk, wv, uk, uv, wo, y, analysis=analysis,
              stop_after=stop_after, rs_chunks=rs_chunks)
    nc.compile()
    return nc


def _body(tc, hT, iT, mT, wq, wk, wv, uk, uv, wo, y, analysis=False,
          stop_after=None, rs_chunks=4):
    nc = tc.nc

    def _finish_early():
        with tc.tile_pool(name="fin", bufs=1) as fin:
            t = fin.tile([P, D], F32, name="fint", tag="fint")
            nc.gpsimd.memset(t, 0.0)
            for mo in range(4):
                nc.sync.dma_start(y[mo * P:(mo + 1) * P, :], t)
    Exp = mybir.ActivationFunctionType.Exp

    from contextlib import ExitStack

    with ExitStack() as ctx:
        # Persistent intermediates (live across phases).
        op = ctx.enter_context(tc.tile_pool(name="op", bufs=1))
        qT = [op.tile([P, S], BF16, name=f"qT{i}", tag=f"qT{i}") for i in range(4)]
        kTt = [op.tile([P, SK], BF16, name=f"kT{i}", tag=f"kT{i}") for i in range(4)]
        vA = [op.tile([P, HL, DH + 1], BF16, name=f"vA{i}", tag=f"vA{i}") for i in range(KT)]
        xT = [op.tile([P, S], BF16, name=f"xT{i}", tag=f"xT{i}") for i in range(4)]

        # All SBUF pools are opened flat (no nested scopes): total SBUF fits,
        # and avoiding cross-phase slot reuse lets the scheduler overlap
        # phases. PSUM pools stay scoped (only 8 banks exist).
        wp = ctx.enter_context(tc.tile_pool(name="wp", bufs=1))
        app = ctx.enter_context(tc.tile_pool(name="ap", bufs=1))
        stg1 = ctx.enter_context(tc.tile_pool(name="stg1", bufs=4))
        mp = ctx.enter_context(tc.tile_pool(name="mp", bufs=1))
        ppool = ctx.enter_context(tc.tile_pool(name="ppool", bufs=4))
        small = ctx.enter_context(tc.tile_pool(name="small", bufs=2))
        wop = ctx.enter_context(tc.tile_pool(name="wop", bufs=1))
        stg = ctx.enter_context(tc.tile_pool(name="stg", bufs=2))
        dp = ctx.enter_context(tc.tile_pool(name="dp", bufs=1, space="DRAM"))

        # ---------------- Phase 1: projections (bf16 matmuls) ----------------
        with tc.tile_pool(name="pp1", bufs=8, space="PSUM") as pp1:

            def alloc_bf(pool, nm, n_tiles, width):
                return [pool.tile([P, width], BF16, name=f"{nm}{k}", tag=f"{nm}{k}")
                        for k in range(n_tiles)]

            def load_one(dram, t, k, width):
                st = stg1.tile([P, S], F32, name="ldstg", tag="ldstg")
                nc.sync.dma_start(st[:, :width], dram[k * P:(k + 1) * P, :])
                nc.vector.tensor_copy(t, st[:, :width])

            hTs = alloc_bf(app, "hTs", 8, S)
            iTs = alloc_bf(app, "iTs", 8, SI)
            wqs = alloc_bf(wp, "wqs", 8, OC)
            wks = alloc_bf(wp, "wks", 8, OC)
            wvs = alloc_bf(wp, "wvs", 8, OC)
            uks = alloc_bf(wp, "uks", 8, OC)
            uvs = alloc_bf(wp, "uvs", 8, OC)

            # Load in first-use order: the first qT matmul only waits on ~1 MB
            # of DMA, and P1's tail is DMA_end + the matmuls gated by the last
            # arrival, so the image-side tensors (smallest matmul groups) and
            # u_v come last.
            for k in range(8):
                load_one(wq, wqs[k], k, OC)
                load_one(hT, hTs[k], k, S)
            for k in range(8):
                load_one(wv, wvs[k], k, OC)
            for k in range(8):
                load_one(wk, wks[k], k, OC)
            for k in range(8):
                load_one(iT, iTs[k], k, SI)
                load_one(uk, uks[k], k, OC)
            for k in range(8):
                load_one(uv, uvs[k], k, OC)

            # qT / kT (transposed layouts): out[m=o_tile, n=s]
            def proj_T(ws, rhs_tiles, nfree, out_fn):
                nn = nfree // 512
                for mo in range(4):
                    for nq in range(nn):
                        ps = pp1.tile([P, 512], F32, name="ps1", tag="ps1")
                        for k in range(8):
                            nc.tensor.matmul(
                                ps,
                                lhsT=ws[k][:, mo * P:(mo + 1) * P],
                                rhs=rhs_tiles[k][:, nq * 512:(nq + 1) * 512],
                                start=(k == 0), stop=(k == 7),
                            )
                        out_fn(mo, nq, ps)

            # v in natural layout [k', o] -> vA tiles, with a ones column per head
            def v_tiles(so_range):
                for so in so_range:
                    ps = pp1.tile([P, 512], F32, name="psv", tag="ps1")
                    for k in range(8):
                        if so < 8:
                            lhsT = hTs[k][:, so * P:(so + 1) * P]
                            rhs = wvs[k]
                        else:
                            lhsT = iTs[k][:, (so - 8) * P:(so - 7) * P]
                            rhs = uvs[k]
                        nc.tensor.matmul(ps, lhsT=lhsT, rhs=rhs,
                                         start=(k == 0), stop=(k == 7))
                    nc.vector.tensor_copy(vA[so][:, :, 0:DH],
                                          ps.rearrange("p (h d) -> p h d", h=HL))
                    nc.gpsimd.memset(vA[so][:, :, DH:DH + 1], 1.0)

            proj_T(wqs, hTs, S, lambda mo, nq, ps: nc.scalar.copy(
                qT[mo][:, nq * 512:(nq + 1) * 512], ps))
            v_tiles(range(8))
            proj_T(wks, hTs, S, lambda mo, nq, ps: nc.scalar.copy(
                kTt[mo][:, nq * 512:(nq + 1) * 512], ps))
            proj_T(uks, iTs, SI, lambda mo, nq, ps: nc.scalar.copy(
                kTt[mo][:, S + nq * 512:S + (nq + 1) * 512], ps))
            v_tiles(range(8, KT))

        if stop_after == "p1":
            _finish_early()
            return

        # ---------------- Phase 2+3: mask + attention ----------------
        # Heads are processed in pairs (2j, 2j+1): their K=64 score matmuls
        # use disjoint PE row groups (base partitions 0 / 64) and distinct
        # PSUM banks, so the hardware runs them concurrently.
        mTs = [mp.tile([P, S], BF16, name=f"mTs{i}", tag=f"mTs{i}") for i in range(KT)]
        for ko in range(KT):
            mstg = stg1.tile([P, S], I32, name="mstg", tag="ldstg")
            nc.sync.dma_start(mstg, mT[ko * P:(ko + 1) * P, :])
            nc.vector.tensor_copy(mTs[ko], mstg)

        # Load + cast w_o early so phase 4 starts without waiting on DMA.
        wo_bf = [wop.tile([P, D], BF16, name=f"wob{k}", tag=f"wob{k}") for k in range(4)]
        for k in range(4):
            ws = stg.tile([P, D], F32, name="wstg", tag="wstg")
            nc.sync.dma_start(ws, wo[k * P:(k + 1) * P, :])
            nc.vector.tensor_copy(wo_bf[k], ws)

        def _emit_xmm(pj, ko, ptA, ptB, xA, xB):
            for xp, pt, hh in ((xA, ptA, 2 * pj), (xB, ptB, 2 * pj + 1)):
                for nq in range(2):
                    nc.tensor.matmul(
                        xp[:, nq * 512:(nq + 1) * 512],
                        lhsT=vA[ko][:, hh, :],
                        rhs=pt[:, nq * 512:(nq + 1) * 512],
                        start=(ko == 0), stop=(ko == KT - 1),
                    )

        with tc.tile_pool(name="spsum", bufs=1, space="PSUM") as spsum, \
             tc.tile_pool(name="xpsum", bufs=1, space="PSUM") as xpsum:
            for pj in range(HL // 2):
                xA = xpsum.tile([DH + 1, S], F32, name="xA", tag="x", bufs=2)
                xB = xpsum.tile([DH + 1, S], F32, name="xB", tag="x", bufs=2)
                prev = None
                for ko in range(KT):
                    spA = spsum.tile([P, S], F32, name="spA", tag="sp", bufs=2)
                    spB = spsum.tile([P, S], F32, name="spB", tag="sp", bufs=2)
                    for sp, p0 in ((spA, 0), (spB, 64)):
                        for nq in range(2):
                            nc.tensor.matmul(
                                sp[:, nq * 512:(nq + 1) * 512],
                                lhsT=kTt[pj][p0:p0 + 64, ko * P:(ko + 1) * P],
                                rhs=qT[pj][p0:p0 + 64, nq * 512:(nq + 1) * 512],
                                start=True, stop=True,
                            )
                    if prev is not None:
                        _emit_xmm(pj, *prev)
                    ptA = ppool.tile([P, S], BF16, name="ptA", tag="ptA")
                    ptB = ppool.tile([P, S], BF16, name="ptB", tag="ptB")
                    nc.scalar.activation(ptA, spA, Exp, scale=0.125)
                    nc.vector.tensor_mul(ptA, ptA, mTs[ko])
                    nc.scalar.activation(ptB, spB, Exp, scale=0.125)
                    nc.vector.tensor_mul(ptB, ptB, mTs[ko])
                    prev = (ko, ptA, ptB, xA, xB)
                _emit_xmm(pj, *prev)
                for xp, p0 in ((xA, 0), (xB, 64)):
                    rs = small.tile([1, S], F32, name="rs", tag="rs")
                    nc.vector.reciprocal(rs, xp[DH:DH + 1, :])
                    rb = small.tile([64, S], F32, name="rb", tag="rb")
                    nc.gpsimd.partition_broadcast(rb, rs)
                    nc.vector.tensor_mul(xT[pj][p0:p0 + 64, :], xp[0:DH, :], rb)

        if stop_after == "attn":
            _finish_early()
            return

        # -------- Phase 4: output projection + chunked ReduceScatter --------
        # Two chunked ReduceScatters so the first overlaps with the second
        # half of the y matmuls. Chunk c holds y-rows [even-core slice c ;
        # odd-core slice c], so RS hands rank0 the even-core rows and rank1
        # the odd-core rows, each landing at local rows [c*256:(c+1)*256].
        with tc.tile_pool(name="yp", bufs=2, space="PSUM") as yp:
            # Partial-y exchange runs in bf16: halves collective bytes; the
            # f32 output is reconstituted from the bf16 pair-sum on device.
            # rs_chunks ReduceScatters pipeline the exchange behind the y
            # matmuls; only the last one's latency is exposed.
            NC_ = rs_chunks                  # chunks
            MPC = 8 // NC_                   # m-tiles per chunk
            RPC = MPC // 2                   # m-tiles per half per chunk
            CROWS = RPC * P                  # local output rows per chunk
            ybounce = [dp.tile([2 * CROWS, D], BF16, name=f"ybounce{c}",
                               tag=f"ybounce{c}") for c in range(NC_)]
            yout = [dp.tile([CROWS, D], BF16, name=f"yout{c}", tag=f"yout{c}")
                    for c in range(NC_)]
            # m-tile mo (y rows mo*128) -> (chunk, position-within-chunk):
            # chunk c = even-half tiles [c*RPC, (c+1)*RPC) ++ odd-half ones.
            chunk_of = {}
            order = []
            for c in range(NC_):
                for r in range(RPC):
                    chunk_of[c * RPC + r] = (c, r)
                    chunk_of[4 + c * RPC + r] = (c, RPC + r)
                order += [c * RPC + r for r in range(RPC)]
                order += [4 + c * RPC + r for r in range(RPC)]

            def rs_chunk(c):
                if not analysis:
                    nc.gpsimd.collective_compute(
                        "ReduceScatter",
                        mybir.AluOpType.add,
                        replica_groups=[[0, 1], [2, 3], [4, 5], [6, 7]],
                        ins=[ybounce[c].opt()],
                        outs=[yout[c].opt()],
                    )
                src = ybounce[c] if analysis else yout[c]
                for t in range(RPC):
                    yrb = stg.tile([P, D], BF16, name="yrb", tag="yrd", bufs=1)
                    nc.sync.dma_start(yrb, src[t * P:(t + 1) * P, :])
                    yfb = stg.tile([P, D], F32, name="yfb", tag="ysb")
                    nc.vector.tensor_copy(yfb, yrb)
                    nc.sync.dma_start(
                        y[c * CROWS + t * P:c * CROWS + (t + 1) * P, :], yfb)

            for i, mo in enumerate(order):
                c, pos = chunk_of[mo]
                yps = yp.tile([P, D], F32, name="yps", tag="yps")
                for k in range(4):
                    for nq in range(2):
                        nc.tensor.matmul(
                            yps[:, nq * 512:(nq + 1) * 512],
                            lhsT=xT[k][:, mo * P:(mo + 1) * P],
                            rhs=wo_bf[k][:, nq * 512:(nq + 1) * 512],
                            start=(k == 0), stop=(k == 3),
                        )
                ysb = stg.tile([P, D], BF16, name="ysbo", tag="yrb")
                if mo % 2 == 0:
                    nc.scalar.copy(ysb, yps)
                else:
                    nc.vector.tensor_copy(ysb, yps)
                nc.sync.dma_start(ybounce[c][pos * P:(pos + 1) * P, :], ysb)
                if i % MPC == MPC - 1 and i != len(order) - 1:
                    rs_chunk(i // MPC)
            rs_chunk(NC_ - 1)


def _get_nc():
    if "nc" not in _CACHE:
        _CACHE["nc"] = _build_nc()
    return _CACHE["nc"]


def make_in_maps(hidden_states, image_hidden_states, attention_mask,
                 w_q, w_k, w_v, u_k, u_v, w_o):
    hidden = np.asarray(hidden_states, dtype=np.float32)
    image = np.asarray(image_hidden_states, dtype=np.float32)
    mask = np.asarray(attention_mask)
    w_q = np.asarray(w_q, dtype=np.float32)
    w_k = np.asarray(w_k, dtype=np.float32)
    w_v = np.asarray(w_v, dtype=np.float32)
    u_k = np.asarray(u_k, dtype=np.float32)
    u_v = np.asarray(u_v, dtype=np.float32)
    w_o = np.asarray(w_o, dtype=np.float32)

    in_maps = []
    for c in range(8):
        b, hg = c // 2, c % 2
        sl = slice(hg * OC, (hg + 1) * OC)
        in_maps.append({
            "hT": np.ascontiguousarray(hidden[b].T),
            "iT": np.ascontiguousarray(image[b].T),
            "mT": np.ascontiguousarray(mask[b, 0].T.astype(np.int32)),
            "wq": np.ascontiguousarray(w_q[sl, :].T),
            "wk": np.ascontiguousarray(w_k[sl, :].T),
            "wv": np.ascontiguousarray(w_v[sl, :].T),
            "uk": np.ascontiguousarray(u_k[sl, :].T),
            "uv": np.ascontiguousarray(u_v[sl, :].T),
            "wo": np.ascontiguousarray(w_o.T[sl, :]),
        })
    return in_maps


def run(in_maps, **kwargs):
    nc = _get_nc()
    return bass_utils.run_bass_kernel_spmd(nc, in_maps, core_ids=list(range(8)),
                                           **kwargs)


def kernel(hidden_states, image_hidden_states, attention_mask,
           w_q, w_k, w_v, u_k, u_v, w_o):
    in_maps = make_in_maps(hidden_states, image_hidden_states, attention_mask,
                           w_q, w_k, w_v, u_k, u_v, w_o)
    res = run(in_maps)
    out = np.empty((4, S, D), dtype=np.float32)
    for b in range(4):
        out[b, 0:S // 2] = res.results[2 * b]["y"]
        out[b, S // 2:S] = res.results[2 * b + 1]["y"]
    return out

